# revision 19
# baseline (speedup 1.0000x reference)
"""Trainium2 Bass kernel for the OFDM channel problem.

Full inputs in, full outputs out; internally data-parallel over the batch
dim N across 8 NeuronCores (2 batches/core). Per core:
  - factor[s, np] = sum_l cof[np,l] * exp(-i * phase_coef[l] * (delay1[np,l] + delay2[s]))
    computed on-chip (ACT Ln/Sin + one PE matmul against a block-diag cof matrix)
  - out[np, s, m, :] = complex_mul(x[np, s, m, :], factor[s, np])
    as 1 ScalarE scale + 2 VectorE scalar_tensor_tensor FMAs per (np) tile
  - H_t[np, m] = sum_l cof[np,l] * exp(-2pi*i*l*m/M)  (16-tap DFT as PE matmul)
"""

import contextlib
import ctypes
import sys
import types

import numpy as np
from contextlib import ExitStack

import concourse.bacc as bacc
import concourse.tile as tile
from concourse import mybir
from concourse import bass_utils
from concourse._compat import axon_active, get_trn_type

# problem constants (hardcoded per harness contract)
N, P, L, M, S = 16, 4, 16, 1024, 128
NCORES = 8
NLOC = N // NCORES          # batches per core
NP = NLOC * P               # (n, p) pairs per core = 8
C_LIGHT = 3.0e8
VELOCITY = 100.0
CARRIER_FREQ = 3.0e9

F32 = mybir.dt.float32
AF = mybir.ActivationFunctionType
OP = mybir.AluOpType

TRACE = False
LAST_RESULT = None

_AXON_SO = "/opt/axon/libaxon_pjrt.so"


def _ensure_axon_hooks():
    """bass_utils' trace=True path imports antenv.axon_hooks, which this
    image's antenv lacks. Provide it, registering an NTFF profile hook
    driven via ctypes against the loaded libaxon_pjrt.so (same scheme the
    axon boot script uses)."""
    try:
        import antenv.axon_hooks  # noqa: F401
        return
    except ImportError:
        pass
    import antenv

    mod = types.ModuleType("antenv.axon_hooks")
    holder = [None]
    mod.set_axon_ntff_profile_hook = lambda h: holder.__setitem__(0, h)
    mod.get_axon_ntff_profile_hook = lambda: holder[0]
    sys.modules["antenv.axon_hooks"] = mod
    antenv.axon_hooks = mod

    try:
        lib = ctypes.CDLL(_AXON_SO)
        if not hasattr(lib, "axon_start_nrt_profile"):
            return
        lib.axon_start_nrt_profile.argtypes = [
            ctypes.POINTER(ctypes.c_int64), ctypes.c_size_t]
        lib.axon_start_nrt_profile.restype = ctypes.c_int64
        lib.axon_stop_nrt_profile.argtypes = [ctypes.c_char_p]
        lib.axon_stop_nrt_profile.restype = ctypes.c_int64

        @contextlib.contextmanager
        def _hook(output_dir, device_ids):
            import jax
            jax.devices()
            if device_ids:
                ids = (ctypes.c_int64 * len(device_ids))(*device_ids)
                rc = lib.axon_start_nrt_profile(ids, len(device_ids))
            else:
                rc = lib.axon_start_nrt_profile(None, 0)
            if rc != 0:
                raise RuntimeError(f"axon_start_nrt_profile rc={rc}")
            try:
                yield
            finally:
                n = lib.axon_stop_nrt_profile(str(output_dir).encode())
                print(f"profile: {n} ntff file(s) written to {output_dir}",
                      file=sys.stderr)

        mod.set_axon_ntff_profile_hook(_hook)
    except OSError:
        pass


_ensure_axon_hooks()


def _host_consts():
    max_doppler = VELOCITY / C_LIGHT * CARRIER_FREQ
    angles = np.linspace(0.0, 2.0 * np.pi, L)
    phase_coef = 2.0 * np.pi * np.cos(angles) * max_doppler            # (L,)
    delay2 = np.linspace(0.0, (S - 1) * (0.0005 / 14.0), S)            # (S,)

    # p2[(j,l), s] = phase_coef[l] * delay2[s], tiled over the NP pairs j
    p2 = np.tile((phase_coef[:, None] * delay2[None, :]), (NP, 1))     # (128, S)
    p2s = p2 / (2.0 * np.pi)                                           # for range reduction
    # kcol[(j,l)] = phase_coef[l] * (-100 / c)   (so kcol * ln(cof) = phase_coef*delay1)
    kcol = np.tile(phase_coef * (-100.0 / C_LIGHT), NP)[:, None]       # (128, 1)
    kcols = kcol / (2.0 * np.pi)
    # block-diagonal selector: mmask[(j,l), j'] = 1 if j == j'
    mmask = np.zeros((NP * L, NP), dtype=np.float64)
    for j in range(NP):
        mmask[j * L:(j + 1) * L, j] = 1.0
    # DFT matrices for the 16-tap frequency response
    lm = np.outer(np.arange(L), np.arange(M)) * (2.0 * np.pi / M)      # (L, M)
    wre = np.cos(lm)
    wim = -np.sin(lm)
    f = np.float32
    # pack the (128, .) constants into one tensor (cof_col column appended
    # per-call) and the (16, .) constants into another, so each core needs
    # only two constant DMA triggers.
    cc = np.concatenate([p2, p2s, kcol, kcols, mmask], axis=1)       # (128, 266)
    wc = np.concatenate([wre, wim], axis=1)                          # (16, 2048)
    return np.ascontiguousarray(cc, f), np.ascontiguousarray(wc, f)


_NC_CACHE = None


def _build():
    global _NC_CACHE
    if _NC_CACHE is not None:
        return _NC_CACHE

    nc = bacc.Bacc(
        get_trn_type() or "TRN2",
        target_bir_lowering=False,
        debug=not axon_active(),
        enable_asserts=False,
        num_devices=NCORES,
    )

    x_ext = nc.dram_tensor("x", [NP, S, M, 2], F32, kind="ExternalInput").ap()
    cc_ext = nc.dram_tensor("cc", [NP * L, 267], F32, kind="ExternalInput").ap()
    wc_ext = nc.dram_tensor("wc", [L, 2 * M + NP], F32, kind="ExternalInput").ap()
    out_ext = nc.dram_tensor("out", [NP, S, M, 2], F32, kind="ExternalOutput").ap()
    hout_ext = nc.dram_tensor("hout", [NP, M, 2], F32, kind="ExternalOutput").ap()

    with tile.TileContext(nc) as tc, ExitStack() as ctx:
        const = ctx.enter_context(tc.tile_pool(name="const", bufs=1))
        fac = ctx.enter_context(tc.tile_pool(name="fac", bufs=1))
        psum = ctx.enter_context(tc.tile_pool(name="psum", bufs=1, space="PSUM"))
        xpool = ctx.enter_context(tc.tile_pool(name="xpool", bufs=8))
        ypool = ctx.enter_context(tc.tile_pool(name="ypool", bufs=4))
        opool = ctx.enter_context(tc.tile_pool(name="opool", bufs=6))

        PART = NP * L  # 128

        # two packed constant tensors -> two DMA triggers, issued first so
        # the factor chain unblocks before the 1MB x tiles saturate HBM
        cc_t = const.tile([PART, 267], F32)
        nc.sync.dma_start(cc_t[:], cc_ext[:])
        wc_t = const.tile([L, 2 * M + NP], F32)
        nc.sync.dma_start(wc_t[:], wc_ext[:])
        p2_t = cc_t[:, 0:S]
        p2s_t = cc_t[:, S:2 * S]
        kcol_t = cc_t[:, 256:257]
        kcols_t = cc_t[:, 257:258]
        mmask_t = cc_t[:, 258:266]
        cofcol_t = cc_t[:, 266:267]
        wre_t = wc_t[:, 0:M]
        wim_t = wc_t[:, M:2 * M]
        coft_t = wc_t[:, 2 * M:2 * M + NP]

        # ---- factor[s, j] = sum_l cof[j,l] * exp(-i * (c1 + p2)) ----
        # phases reach +-29 rad; HW Sin is only valid on ~[-pi, pi], so
        # range-reduce with the magic-number round: k = round(x/2pi), y = x - 2pi*k.
        TWO_PI = float(2.0 * np.pi)
        MAGIC = float(1.5 * 2 ** 23)

        lncof = fac.tile([PART, 1], F32)
        nc.scalar.activation(lncof[:], cofcol_t, AF.Ln)
        c1 = fac.tile([PART, 1], F32)                       # phase_coef[l] * delay1
        nc.vector.tensor_mul(c1[:], lncof[:], kcol_t)
        c1s = fac.tile([PART, 1], F32)                      # c1 / 2pi
        nc.vector.tensor_mul(c1s[:], lncof[:], kcols_t)
        c1p = fac.tile([PART, 1], F32)                      # + pi/2 for cos via Sin
        nc.vector.tensor_scalar_add(c1p[:], c1[:], float(np.pi / 2.0))
        c1ps = fac.tile([PART, 1], F32)
        nc.vector.tensor_scalar_add(c1ps[:], c1s[:], 0.25)
        zerob = fac.tile([PART, 1], F32)
        nc.vector.memset(zerob[:], 0.0)

        sinp = fac.tile([PART, S], F32)                     # sin(phases)
        cosp = fac.tile([PART, S], F32)                     # cos(phases)
        for idx, (trig_out, cb, cbs) in enumerate(((sinp, c1, c1s), (cosp, c1p, c1ps))):
            x_ph = fac.tile([PART, S], F32, tag=f"xph{idx}")
            nc.vector.tensor_scalar(x_ph[:], p2_t, cb[:], None, op0=OP.add)
            u_ph = fac.tile([PART, S], F32, tag=f"uph{idx}")
            nc.vector.tensor_scalar(u_ph[:], p2s_t, cbs[:], MAGIC,
                                    op0=OP.add, op1=OP.add)
            k_ph = fac.tile([PART, S], F32, tag=f"kph{idx}")
            nc.vector.tensor_scalar(k_ph[:], u_ph[:], -MAGIC, None, op0=OP.add)
            y_ph = fac.tile([PART, S], F32, tag=f"yph{idx}")
            nc.vector.scalar_tensor_tensor(y_ph[:], k_ph[:], -TWO_PI, x_ph[:],
                                           op0=OP.mult, op1=OP.add)
            nc.scalar.activation(trig_out[:], y_ph[:], AF.Sin,
                                 bias=zerob[:], scale=1.0)

        cof_bd = fac.tile([PART, NP], F32)                  # block-diag cof
        nc.vector.tensor_scalar_mul(cof_bd[:], mmask_t, cofcol_t)

        fre_ps = psum.tile([PART, NP], F32)
        nc.tensor.matmul(fre_ps[:], cosp[:], cof_bd[:], start=True, stop=True)
        fsin_ps = psum.tile([PART, NP], F32)
        nc.tensor.matmul(fsin_ps[:], sinp[:], cof_bd[:], start=True, stop=True)

        fre = fac.tile([PART, NP], F32)                     # Re(factor)[s, j]
        nc.vector.tensor_copy(fre[:], fre_ps[:])
        fimneg = fac.tile([PART, NP], F32)                  # -Im(factor)[s, j]
        nc.vector.tensor_copy(fimneg[:], fsin_ps[:])
        fim = fac.tile([PART, NP], F32)                     # Im(factor)[s, j]
        nc.vector.tensor_scalar_mul(fim[:], fsin_ps[:], -1.0)

        # ---- H_t = 16-tap DFT of cof ----
        hre_ps = psum.tile([NP, M], F32)
        him_ps = psum.tile([NP, M], F32)
        for h in range(2):
            sl = slice(h * 512, (h + 1) * 512)
            nc.tensor.matmul(hre_ps[:, sl], coft_t, wre_t[:, sl.start:sl.stop], start=True, stop=True)
            nc.tensor.matmul(him_ps[:, sl], coft_t, wim_t[:, sl.start:sl.stop], start=True, stop=True)
        hout_t = fac.tile([NP, M, 2], F32)
        nc.vector.tensor_copy(hout_t[:, :, 0], hre_ps[:])
        nc.vector.tensor_copy(hout_t[:, :, 1], him_ps[:])
        nc.gpsimd.dma_start(hout_ext[:], hout_t[:])

        # ---- main elementwise complex multiply ----
        # inputs issue from sync (HWDGE), outputs from gpsimd (SWDGE) so the
        # two DMA streams don't head-of-line block each other's sequencer
        for j in range(NP):
            x_t = xpool.tile([S, M, 2], F32)
            nc.sync.dma_start(x_t[:], x_ext[j])
            # the last pairs are the pipeline tail: process them in half-m
            # chunks so the final compute + output drain is finer-grained
            nch = 2 if j >= NP - 2 else 1
            MH = M // nch
            for h in range(nch):
                msl = slice(h * MH, (h + 1) * MH)
                y_t = ypool.tile([S, MH, 2], F32, tag=f"y{nch}")
                # y = x * Re(factor)  (both ri lanes, per-partition scale)
                nc.scalar.activation(y_t[:], x_t[:, msl, :], AF.Copy,
                                     scale=fre[:, j:j + 1])
                o_t = opool.tile([S, MH, 2], F32, tag=f"o{nch}")
                # out_r = xi * (-fim) + y_r ;  out_i = xr * fim + y_i
                nc.vector.scalar_tensor_tensor(
                    o_t[:, :, 0], x_t[:, msl, 1], fimneg[:, j:j + 1], y_t[:, :, 0],
                    op0=OP.mult, op1=OP.add)
                nc.vector.scalar_tensor_tensor(
                    o_t[:, :, 1], x_t[:, msl, 0], fim[:, j:j + 1], y_t[:, :, 1],
                    op0=OP.mult, op1=OP.add)
                nc.gpsimd.dma_start(out_ext[j, :, msl, :], o_t[:])

    nc.compile()
    _NC_CACHE = nc
    return nc


def kernel(input_ri, cof, Ns):
    global LAST_RESULT
    assert int(Ns) == S, f"kernel hardcodes S={S}, got Ns={Ns}"
    input_ri = np.ascontiguousarray(np.asarray(input_ri, dtype=np.float32))
    cof = np.ascontiguousarray(np.asarray(cof, dtype=np.float32))
    assert input_ri.shape == (N, P, S * M, 2) and cof.shape == (N, P, L)

    cc_base, wc_base = _host_consts()
    nc = _build()

    in_maps = []
    for c in range(NCORES):
        n0 = c * NLOC
        x_sh = np.ascontiguousarray(
            input_ri[n0:n0 + NLOC].reshape(NP, S, M, 2))
        cof_sh = np.ascontiguousarray(cof[n0:n0 + NLOC].reshape(NP, L))
        cc = np.concatenate([cc_base, cof_sh.reshape(NP * L, 1)], axis=1)
        wc = np.concatenate([wc_base, cof_sh.T], axis=1)
        in_maps.append({
            "x": x_sh,
            "cc": np.ascontiguousarray(cc),
            "wc": np.ascontiguousarray(wc),
        })

    res = bass_utils.run_bass_kernel_spmd(
        nc, in_maps, core_ids=list(range(NCORES)), trace=TRACE)
    LAST_RESULT = res

    out_ri = np.concatenate(
        [res.results[c]["out"].reshape(NLOC, P, S * M, 2) for c in range(NCORES)],
        axis=0)
    h_ri = np.concatenate(
        [res.results[c]["hout"].reshape(NLOC, P, M, 2) for c in range(NCORES)],
        axis=0)
    return out_ri, h_ri


# revision 20
# speedup vs baseline: 1.1867x; 1.1867x over previous
"""Trainium2 Bass kernel for the OFDM channel problem.

Full inputs in, full outputs out; internally data-parallel over the batch
dim N across 8 NeuronCores (2 batches/core). Per core:
  - factor[s, np] = sum_l cof[np,l] * exp(-i * phase_coef[l] * (delay1[np,l] + delay2[s]))
    computed on-chip (ACT Ln/Sin + one PE matmul against a block-diag cof matrix)
  - out[np, s, m, :] = complex_mul(x[np, s, m, :], factor[s, np])
    as 1 ScalarE scale + 2 VectorE scalar_tensor_tensor FMAs per (np) tile
  - H_t[np, m] = sum_l cof[np,l] * exp(-2pi*i*l*m/M)  (16-tap DFT as PE matmul)
"""

import contextlib
import ctypes
import sys
import types

import numpy as np
from contextlib import ExitStack

import concourse.bacc as bacc
import concourse.tile as tile
from concourse import mybir
from concourse import bass_utils
from concourse._compat import axon_active, get_trn_type

# problem constants (hardcoded per harness contract)
N, P, L, M, S = 16, 4, 16, 1024, 128
NCORES = 8
NLOC = N // NCORES          # batches per core
NP = NLOC * P               # (n, p) pairs per core = 8
C_LIGHT = 3.0e8
VELOCITY = 100.0
CARRIER_FREQ = 3.0e9

F32 = mybir.dt.float32
AF = mybir.ActivationFunctionType
OP = mybir.AluOpType

TRACE = False
LAST_RESULT = None

_AXON_SO = "/opt/axon/libaxon_pjrt.so"


def _ensure_axon_hooks():
    """bass_utils' trace=True path imports antenv.axon_hooks, which this
    image's antenv lacks. Provide it, registering an NTFF profile hook
    driven via ctypes against the loaded libaxon_pjrt.so (same scheme the
    axon boot script uses)."""
    try:
        import antenv.axon_hooks  # noqa: F401
        return
    except ImportError:
        pass
    import antenv

    mod = types.ModuleType("antenv.axon_hooks")
    holder = [None]
    mod.set_axon_ntff_profile_hook = lambda h: holder.__setitem__(0, h)
    mod.get_axon_ntff_profile_hook = lambda: holder[0]
    sys.modules["antenv.axon_hooks"] = mod
    antenv.axon_hooks = mod

    try:
        lib = ctypes.CDLL(_AXON_SO)
        if not hasattr(lib, "axon_start_nrt_profile"):
            return
        lib.axon_start_nrt_profile.argtypes = [
            ctypes.POINTER(ctypes.c_int64), ctypes.c_size_t]
        lib.axon_start_nrt_profile.restype = ctypes.c_int64
        lib.axon_stop_nrt_profile.argtypes = [ctypes.c_char_p]
        lib.axon_stop_nrt_profile.restype = ctypes.c_int64

        @contextlib.contextmanager
        def _hook(output_dir, device_ids):
            import jax
            jax.devices()
            if device_ids:
                ids = (ctypes.c_int64 * len(device_ids))(*device_ids)
                rc = lib.axon_start_nrt_profile(ids, len(device_ids))
            else:
                rc = lib.axon_start_nrt_profile(None, 0)
            if rc != 0:
                raise RuntimeError(f"axon_start_nrt_profile rc={rc}")
            try:
                yield
            finally:
                n = lib.axon_stop_nrt_profile(str(output_dir).encode())
                print(f"profile: {n} ntff file(s) written to {output_dir}",
                      file=sys.stderr)

        mod.set_axon_ntff_profile_hook(_hook)
    except OSError:
        pass


_ensure_axon_hooks()


def _host_consts():
    max_doppler = VELOCITY / C_LIGHT * CARRIER_FREQ
    angles = np.linspace(0.0, 2.0 * np.pi, L)
    phase_coef = 2.0 * np.pi * np.cos(angles) * max_doppler            # (L,)
    delay2 = np.linspace(0.0, (S - 1) * (0.0005 / 14.0), S)            # (S,)

    # p2[(j,l), s] = phase_coef[l] * delay2[s], tiled over the NP pairs j
    p2 = np.tile((phase_coef[:, None] * delay2[None, :]), (NP, 1))     # (128, S)
    p2s = p2 / (2.0 * np.pi)                                           # for range reduction
    # kcol[(j,l)] = phase_coef[l] * (-100 / c)   (so kcol * ln(cof) = phase_coef*delay1)
    kcol = np.tile(phase_coef * (-100.0 / C_LIGHT), NP)[:, None]       # (128, 1)
    kcols = kcol / (2.0 * np.pi)
    # block-diagonal selector: mmask[(j,l), j'] = 1 if j == j'
    mmask = np.zeros((NP * L, NP), dtype=np.float64)
    for j in range(NP):
        mmask[j * L:(j + 1) * L, j] = 1.0
    # DFT matrices for the 16-tap frequency response
    lm = np.outer(np.arange(L), np.arange(M)) * (2.0 * np.pi / M)      # (L, M)
    wre = np.cos(lm)
    wim = -np.sin(lm)
    f = np.float32
    # pack the (128, .) constants into one tensor (cof_col column appended
    # per-call) and the (16, .) constants into another, so each core needs
    # only two constant DMA triggers.
    cc = np.concatenate([p2, p2s, kcol, kcols, mmask], axis=1)       # (128, 266)
    wc = np.concatenate([wre, wim], axis=1)                          # (16, 2048)
    return np.ascontiguousarray(cc, f), np.ascontiguousarray(wc, f)


_NC_CACHE = None


def _build():
    global _NC_CACHE
    if _NC_CACHE is not None:
        return _NC_CACHE

    nc = bacc.Bacc(
        get_trn_type() or "TRN2",
        target_bir_lowering=False,
        debug=not axon_active(),
        enable_asserts=False,
        num_devices=NCORES,
    )

    x_ext = nc.dram_tensor("x", [NP, S, M, 2], F32, kind="ExternalInput").ap()
    cc_ext = nc.dram_tensor("cc", [NP * L, 267], F32, kind="ExternalInput").ap()
    wc_ext = nc.dram_tensor("wc", [L, 2 * M + NP], F32, kind="ExternalInput").ap()
    out_ext = nc.dram_tensor("out", [NP, S, M, 2], F32, kind="ExternalOutput").ap()
    hout_ext = nc.dram_tensor("hout", [NP, M, 2], F32, kind="ExternalOutput").ap()

    with tile.TileContext(nc) as tc, ExitStack() as ctx:
        const = ctx.enter_context(tc.tile_pool(name="const", bufs=1))
        fac = ctx.enter_context(tc.tile_pool(name="fac", bufs=1))
        psum = ctx.enter_context(tc.tile_pool(name="psum", bufs=1, space="PSUM"))
        xpool = ctx.enter_context(tc.tile_pool(name="xpool", bufs=8))
        ypool = ctx.enter_context(tc.tile_pool(name="ypool", bufs=4))
        opool = ctx.enter_context(tc.tile_pool(name="opool", bufs=6))

        PART = NP * L  # 128

        # two packed constant tensors -> two DMA triggers, issued first so
        # the factor chain unblocks before the 1MB x tiles saturate HBM
        cc_t = const.tile([PART, 267], F32)
        nc.sync.dma_start(cc_t[:], cc_ext[:])
        wc_t = const.tile([L, 2 * M + NP], F32)
        nc.sync.dma_start(wc_t[:], wc_ext[:])
        p2_t = cc_t[:, 0:S]
        p2s_t = cc_t[:, S:2 * S]
        kcol_t = cc_t[:, 256:257]
        kcols_t = cc_t[:, 257:258]
        mmask_t = cc_t[:, 258:266]
        cofcol_t = cc_t[:, 266:267]
        wre_t = wc_t[:, 0:M]
        wim_t = wc_t[:, M:2 * M]
        coft_t = wc_t[:, 2 * M:2 * M + NP]

        # ---- factor[s, j] = sum_l cof[j,l] * exp(-i * (c1 + p2)) ----
        # phases reach +-29 rad; HW Sin is only valid on ~[-pi, pi], so
        # range-reduce with the magic-number round: k = round(x/2pi), y = x - 2pi*k.
        TWO_PI = float(2.0 * np.pi)
        MAGIC = float(1.5 * 2 ** 23)

        lncof = fac.tile([PART, 1], F32)
        nc.scalar.activation(lncof[:], cofcol_t, AF.Ln)
        c1 = fac.tile([PART, 1], F32)                       # phase_coef[l] * delay1
        nc.vector.tensor_mul(c1[:], lncof[:], kcol_t)
        c1s = fac.tile([PART, 1], F32)                      # c1 / 2pi
        nc.vector.tensor_mul(c1s[:], lncof[:], kcols_t)
        c1p = fac.tile([PART, 1], F32)                      # + pi/2 for cos via Sin
        nc.vector.tensor_scalar_add(c1p[:], c1[:], float(np.pi / 2.0))
        c1ps = fac.tile([PART, 1], F32)
        nc.vector.tensor_scalar_add(c1ps[:], c1s[:], 0.25)
        zerob = fac.tile([PART, 1], F32)
        nc.vector.memset(zerob[:], 0.0)

        sinp = fac.tile([PART, S], F32)                     # sin(phases)
        cosp = fac.tile([PART, S], F32)                     # cos(phases)
        for idx, (trig_out, cb, cbs) in enumerate(((sinp, c1, c1s), (cosp, c1p, c1ps))):
            x_ph = fac.tile([PART, S], F32, tag=f"xph{idx}")
            nc.vector.tensor_scalar(x_ph[:], p2_t, cb[:], None, op0=OP.add)
            u_ph = fac.tile([PART, S], F32, tag=f"uph{idx}")
            nc.vector.tensor_scalar(u_ph[:], p2s_t, cbs[:], MAGIC,
                                    op0=OP.add, op1=OP.add)
            k_ph = fac.tile([PART, S], F32, tag=f"kph{idx}")
            nc.vector.tensor_scalar(k_ph[:], u_ph[:], -MAGIC, None, op0=OP.add)
            y_ph = fac.tile([PART, S], F32, tag=f"yph{idx}")
            nc.vector.scalar_tensor_tensor(y_ph[:], k_ph[:], -TWO_PI, x_ph[:],
                                           op0=OP.mult, op1=OP.add)
            nc.scalar.activation(trig_out[:], y_ph[:], AF.Sin,
                                 bias=zerob[:], scale=1.0)

        cof_bd = fac.tile([PART, NP], F32)                  # block-diag cof
        nc.vector.tensor_scalar_mul(cof_bd[:], mmask_t, cofcol_t)

        fre_ps = psum.tile([PART, NP], F32)
        nc.tensor.matmul(fre_ps[:], cosp[:], cof_bd[:], start=True, stop=True)
        fsin_ps = psum.tile([PART, NP], F32)
        nc.tensor.matmul(fsin_ps[:], sinp[:], cof_bd[:], start=True, stop=True)

        fre = fac.tile([PART, NP], F32)                     # Re(factor)[s, j]
        nc.vector.tensor_copy(fre[:], fre_ps[:])
        fimneg = fac.tile([PART, NP], F32)                  # -Im(factor)[s, j]
        nc.vector.tensor_copy(fimneg[:], fsin_ps[:])
        fim = fac.tile([PART, NP], F32)                     # Im(factor)[s, j]
        nc.vector.tensor_scalar_mul(fim[:], fsin_ps[:], -1.0)

        # ---- H_t = 16-tap DFT of cof ----
        hre_ps = psum.tile([NP, M], F32)
        him_ps = psum.tile([NP, M], F32)
        for h in range(2):
            sl = slice(h * 512, (h + 1) * 512)
            nc.tensor.matmul(hre_ps[:, sl], coft_t, wre_t[:, sl.start:sl.stop], start=True, stop=True)
            nc.tensor.matmul(him_ps[:, sl], coft_t, wim_t[:, sl.start:sl.stop], start=True, stop=True)
        hout_t = fac.tile([NP, M, 2], F32)
        nc.vector.tensor_copy(hout_t[:, :, 0], hre_ps[:])
        nc.vector.tensor_copy(hout_t[:, :, 1], him_ps[:])
        nc.gpsimd.dma_start(hout_ext[:], hout_t[:])

        # ---- main elementwise complex multiply ----
        # inputs issue from sync (HWDGE), outputs from gpsimd (SWDGE) so the
        # two DMA streams don't head-of-line block each other's sequencer
        for j in range(NP):
            x_t = xpool.tile([S, M, 2], F32)
            nc.sync.dma_start(x_t[:], x_ext[j])
            y_t = ypool.tile([S, M, 2], F32)
            # y = x * Re(factor)  (both ri lanes, per-partition scale)
            nc.scalar.activation(y_t[:], x_t[:], AF.Copy, scale=fre[:, j:j + 1])
            o_t = opool.tile([S, M, 2], F32)
            # out_r = xi * (-fim) + y_r ;  out_i = xr * fim + y_i
            nc.vector.scalar_tensor_tensor(
                o_t[:, :, 0], x_t[:, :, 1], fimneg[:, j:j + 1], y_t[:, :, 0],
                op0=OP.mult, op1=OP.add)
            nc.vector.scalar_tensor_tensor(
                o_t[:, :, 1], x_t[:, :, 0], fim[:, j:j + 1], y_t[:, :, 1],
                op0=OP.mult, op1=OP.add)
            nc.gpsimd.dma_start(out_ext[j], o_t[:])

    nc.compile()
    _NC_CACHE = nc
    return nc


def kernel(input_ri, cof, Ns):
    global LAST_RESULT
    assert int(Ns) == S, f"kernel hardcodes S={S}, got Ns={Ns}"
    input_ri = np.ascontiguousarray(np.asarray(input_ri, dtype=np.float32))
    cof = np.ascontiguousarray(np.asarray(cof, dtype=np.float32))
    assert input_ri.shape == (N, P, S * M, 2) and cof.shape == (N, P, L)

    cc_base, wc_base = _host_consts()
    nc = _build()

    in_maps = []
    for c in range(NCORES):
        n0 = c * NLOC
        x_sh = np.ascontiguousarray(
            input_ri[n0:n0 + NLOC].reshape(NP, S, M, 2))
        cof_sh = np.ascontiguousarray(cof[n0:n0 + NLOC].reshape(NP, L))
        cc = np.concatenate([cc_base, cof_sh.reshape(NP * L, 1)], axis=1)
        wc = np.concatenate([wc_base, cof_sh.T], axis=1)
        in_maps.append({
            "x": x_sh,
            "cc": np.ascontiguousarray(cc),
            "wc": np.ascontiguousarray(wc),
        })

    res = bass_utils.run_bass_kernel_spmd(
        nc, in_maps, core_ids=list(range(NCORES)), trace=TRACE)
    LAST_RESULT = res

    out_ri = np.concatenate(
        [res.results[c]["out"].reshape(NLOC, P, S * M, 2) for c in range(NCORES)],
        axis=0)
    h_ri = np.concatenate(
        [res.results[c]["hout"].reshape(NLOC, P, M, 2) for c in range(NCORES)],
        axis=0)
    return out_ri, h_ri


# revision 24
# speedup vs baseline: 1.1938x; 1.0060x over previous
"""Trainium2 Bass kernel for the OFDM channel problem.

Full inputs in, full outputs out; internally data-parallel over the batch
dim N across 8 NeuronCores (2 batches/core). Per core:
  - factor[s, np] = sum_l cof[np,l] * exp(-i * phase_coef[l] * (delay1[np,l] + delay2[s]))
    computed on-chip (ACT Ln/Sin + one PE matmul against a block-diag cof matrix)
  - out[np, s, m, :] = complex_mul(x[np, s, m, :], factor[s, np])
    as 1 ScalarE scale + 2 VectorE scalar_tensor_tensor FMAs per (np) tile
  - H_t[np, m] = sum_l cof[np,l] * exp(-2pi*i*l*m/M)  (16-tap DFT as PE matmul)
"""

import contextlib
import ctypes
import sys
import types

import numpy as np
from contextlib import ExitStack

import concourse.bacc as bacc
import concourse.tile as tile
from concourse import mybir
from concourse import bass_utils
from concourse._compat import axon_active, get_trn_type

# problem constants (hardcoded per harness contract)
N, P, L, M, S = 16, 4, 16, 1024, 128
NCORES = 8
NLOC = N // NCORES          # batches per core
NP = NLOC * P               # (n, p) pairs per core = 8
C_LIGHT = 3.0e8
VELOCITY = 100.0
CARRIER_FREQ = 3.0e9

F32 = mybir.dt.float32
AF = mybir.ActivationFunctionType
OP = mybir.AluOpType

TRACE = False
LAST_RESULT = None

_AXON_SO = "/opt/axon/libaxon_pjrt.so"


def _ensure_axon_hooks():
    """bass_utils' trace=True path imports antenv.axon_hooks, which this
    image's antenv lacks. Provide it, registering an NTFF profile hook
    driven via ctypes against the loaded libaxon_pjrt.so (same scheme the
    axon boot script uses)."""
    try:
        import antenv.axon_hooks  # noqa: F401
        return
    except ImportError:
        pass
    import antenv

    mod = types.ModuleType("antenv.axon_hooks")
    holder = [None]
    mod.set_axon_ntff_profile_hook = lambda h: holder.__setitem__(0, h)
    mod.get_axon_ntff_profile_hook = lambda: holder[0]
    sys.modules["antenv.axon_hooks"] = mod
    antenv.axon_hooks = mod

    try:
        lib = ctypes.CDLL(_AXON_SO)
        if not hasattr(lib, "axon_start_nrt_profile"):
            return
        lib.axon_start_nrt_profile.argtypes = [
            ctypes.POINTER(ctypes.c_int64), ctypes.c_size_t]
        lib.axon_start_nrt_profile.restype = ctypes.c_int64
        lib.axon_stop_nrt_profile.argtypes = [ctypes.c_char_p]
        lib.axon_stop_nrt_profile.restype = ctypes.c_int64

        @contextlib.contextmanager
        def _hook(output_dir, device_ids):
            import jax
            jax.devices()
            if device_ids:
                ids = (ctypes.c_int64 * len(device_ids))(*device_ids)
                rc = lib.axon_start_nrt_profile(ids, len(device_ids))
            else:
                rc = lib.axon_start_nrt_profile(None, 0)
            if rc != 0:
                raise RuntimeError(f"axon_start_nrt_profile rc={rc}")
            try:
                yield
            finally:
                n = lib.axon_stop_nrt_profile(str(output_dir).encode())
                print(f"profile: {n} ntff file(s) written to {output_dir}",
                      file=sys.stderr)

        mod.set_axon_ntff_profile_hook(_hook)
    except OSError:
        pass


_ensure_axon_hooks()


def _host_consts():
    max_doppler = VELOCITY / C_LIGHT * CARRIER_FREQ
    angles = np.linspace(0.0, 2.0 * np.pi, L)
    phase_coef = 2.0 * np.pi * np.cos(angles) * max_doppler            # (L,)
    delay2 = np.linspace(0.0, (S - 1) * (0.0005 / 14.0), S)            # (S,)

    # p2[(j,l), s] = phase_coef[l] * delay2[s], tiled over the NP pairs j
    p2 = np.tile((phase_coef[:, None] * delay2[None, :]), (NP, 1))     # (128, S)
    p2s = p2 / (2.0 * np.pi)                                           # for range reduction
    # kcol[(j,l)] = phase_coef[l] * (-100 / c)   (so kcol * ln(cof) = phase_coef*delay1)
    kcol = np.tile(phase_coef * (-100.0 / C_LIGHT), NP)[:, None]       # (128, 1)
    kcols = kcol / (2.0 * np.pi)
    # block-diagonal selector: mmask[(j,l), j'] = 1 if j == j'
    mmask = np.zeros((NP * L, NP), dtype=np.float64)
    for j in range(NP):
        mmask[j * L:(j + 1) * L, j] = 1.0
    # DFT matrices for the 16-tap frequency response
    lm = np.outer(np.arange(L), np.arange(M)) * (2.0 * np.pi / M)      # (L, M)
    wre = np.cos(lm)
    wim = -np.sin(lm)
    f = np.float32
    # pack the (128, .) constants into one tensor (cof_col column appended
    # per-call) and the (16, .) constants into another, so each core needs
    # only two constant DMA triggers.
    cc = np.concatenate([p2, p2s, kcol, kcols, mmask], axis=1)       # (128, 266)
    wc = np.concatenate([wre, wim], axis=1)                          # (16, 2048)
    return np.ascontiguousarray(cc, f), np.ascontiguousarray(wc, f)


_NC_CACHE = None


def _build():
    global _NC_CACHE
    if _NC_CACHE is not None:
        return _NC_CACHE

    nc = bacc.Bacc(
        get_trn_type() or "TRN2",
        target_bir_lowering=False,
        debug=not axon_active(),
        enable_asserts=False,
        num_devices=NCORES,
    )

    x_ext = nc.dram_tensor("x", [NP, S, M, 2], F32, kind="ExternalInput").ap()
    cc_ext = nc.dram_tensor("cc", [NP * L, 267], F32, kind="ExternalInput").ap()
    wc_ext = nc.dram_tensor("wc", [L, 2 * M + NP], F32, kind="ExternalInput").ap()
    out_ext = nc.dram_tensor("out", [NP, S, M, 2], F32, kind="ExternalOutput").ap()
    hout_ext = nc.dram_tensor("hout", [NP, M, 2], F32, kind="ExternalOutput").ap()

    with tile.TileContext(nc) as tc, ExitStack() as ctx:
        const = ctx.enter_context(tc.tile_pool(name="const", bufs=1))
        fac = ctx.enter_context(tc.tile_pool(name="fac", bufs=1))
        psum = ctx.enter_context(tc.tile_pool(name="psum", bufs=1, space="PSUM"))
        xpool = ctx.enter_context(tc.tile_pool(name="xpool", bufs=8))
        ypool = ctx.enter_context(tc.tile_pool(name="ypool", bufs=4))
        opool = ctx.enter_context(tc.tile_pool(name="opool", bufs=6))

        PART = NP * L  # 128

        # two packed constant tensors -> two DMA triggers, issued first so
        # the factor chain unblocks before the 1MB x tiles saturate HBM
        cc_t = const.tile([PART, 267], F32)
        nc.sync.dma_start(cc_t[:], cc_ext[:])
        wc_t = const.tile([L, 2 * M + NP], F32)
        nc.sync.dma_start(wc_t[:], wc_ext[:])
        p2_t = cc_t[:, 0:S]
        p2s_t = cc_t[:, S:2 * S]
        kcol_t = cc_t[:, 256:257]
        kcols_t = cc_t[:, 257:258]
        mmask_t = cc_t[:, 258:266]
        cofcol_t = cc_t[:, 266:267]
        wre_t = wc_t[:, 0:M]
        wim_t = wc_t[:, M:2 * M]
        coft_t = wc_t[:, 2 * M:2 * M + NP]

        # ---- factor[s, j] = sum_l cof[j,l] * exp(-i * (c1 + p2)) ----
        # phases reach +-29 rad; HW Sin is only valid on ~[-pi, pi], so
        # range-reduce with the magic-number round: k = round(x/2pi), y = x - 2pi*k.
        TWO_PI = float(2.0 * np.pi)
        MAGIC = float(1.5 * 2 ** 23)

        lncof = fac.tile([PART, 1], F32)
        nc.scalar.activation(lncof[:], cofcol_t, AF.Ln)
        c1 = fac.tile([PART, 1], F32)                       # phase_coef[l] * delay1
        nc.vector.tensor_mul(c1[:], lncof[:], kcol_t)
        c1s = fac.tile([PART, 1], F32)                      # c1 / 2pi
        nc.vector.tensor_mul(c1s[:], lncof[:], kcols_t)
        c1p = fac.tile([PART, 1], F32)                      # + pi/2 for cos via Sin
        nc.vector.tensor_scalar_add(c1p[:], c1[:], float(np.pi / 2.0))
        c1ps = fac.tile([PART, 1], F32)
        nc.vector.tensor_scalar_add(c1ps[:], c1s[:], 0.25)
        zerob = fac.tile([PART, 1], F32)
        nc.vector.memset(zerob[:], 0.0)

        sinp = fac.tile([PART, S], F32)                     # sin(phases)
        cosp = fac.tile([PART, S], F32)                     # cos(phases)
        for idx, (trig_out, cb, cbs) in enumerate(((sinp, c1, c1s), (cosp, c1p, c1ps))):
            x_ph = fac.tile([PART, S], F32, tag=f"xph{idx}")
            nc.vector.tensor_scalar(x_ph[:], p2_t, cb[:], None, op0=OP.add)
            u_ph = fac.tile([PART, S], F32, tag=f"uph{idx}")
            nc.vector.tensor_scalar(u_ph[:], p2s_t, cbs[:], MAGIC,
                                    op0=OP.add, op1=OP.add)
            k_ph = fac.tile([PART, S], F32, tag=f"kph{idx}")
            nc.vector.tensor_scalar(k_ph[:], u_ph[:], -MAGIC, None, op0=OP.add)
            y_ph = fac.tile([PART, S], F32, tag=f"yph{idx}")
            nc.vector.scalar_tensor_tensor(y_ph[:], k_ph[:], -TWO_PI, x_ph[:],
                                           op0=OP.mult, op1=OP.add)
            nc.scalar.activation(trig_out[:], y_ph[:], AF.Sin,
                                 bias=zerob[:], scale=1.0)

        cof_bd = fac.tile([PART, NP], F32)                  # block-diag cof
        nc.vector.tensor_scalar_mul(cof_bd[:], mmask_t, cofcol_t)

        fre_ps = psum.tile([PART, NP], F32)
        nc.tensor.matmul(fre_ps[:], cosp[:], cof_bd[:], start=True, stop=True)
        fsin_ps = psum.tile([PART, NP], F32)
        nc.tensor.matmul(fsin_ps[:], sinp[:], cof_bd[:], start=True, stop=True)

        fre = fac.tile([PART, NP], F32)                     # Re(factor)[s, j]
        nc.vector.tensor_copy(fre[:], fre_ps[:])
        fimneg = fac.tile([PART, NP], F32)                  # -Im(factor)[s, j]
        nc.vector.tensor_copy(fimneg[:], fsin_ps[:])
        fim = fac.tile([PART, NP], F32)                     # Im(factor)[s, j]
        nc.vector.tensor_scalar_mul(fim[:], fsin_ps[:], -1.0)

        # ---- H_t = 16-tap DFT of cof ----
        hre_ps = psum.tile([NP, M], F32)
        him_ps = psum.tile([NP, M], F32)
        for h in range(2):
            sl = slice(h * 512, (h + 1) * 512)
            nc.tensor.matmul(hre_ps[:, sl], coft_t, wre_t[:, sl.start:sl.stop], start=True, stop=True)
            nc.tensor.matmul(him_ps[:, sl], coft_t, wim_t[:, sl.start:sl.stop], start=True, stop=True)
        hout_t = fac.tile([NP, M, 2], F32)
        nc.vector.tensor_copy(hout_t[:, :, 0], hre_ps[:])
        nc.vector.tensor_copy(hout_t[:, :, 1], him_ps[:])
        nc.gpsimd.dma_start(hout_ext[:], hout_t[:])

        # ---- main elementwise complex multiply ----
        # inputs issue from sync (HWDGE), outputs from gpsimd (SWDGE) so the
        # two DMA streams don't head-of-line block each other's sequencer
        for j in range(NP):
            x_t = xpool.tile([S, M, 2], F32)
            nc.sync.dma_start(x_t[:], x_ext[j])
            t_t = ypool.tile([S, M, 2], F32)
            # cross terms on ACT: t_r = xi * (-fim), t_i = xr * fim
            nc.scalar.activation(t_t[:, :, 0], x_t[:, :, 1], AF.Copy,
                                 scale=fimneg[:, j:j + 1])
            nc.scalar.activation(t_t[:, :, 1], x_t[:, :, 0], AF.Copy,
                                 scale=fim[:, j:j + 1])
            o_t = opool.tile([S, M, 2], F32)
            # out = x * fre + t  (one full-width unit-stride DVE FMA)
            nc.vector.scalar_tensor_tensor(
                o_t[:], x_t[:], fre[:, j:j + 1], t_t[:],
                op0=OP.mult, op1=OP.add)
            nc.gpsimd.dma_start(out_ext[j], o_t[:])

    nc.compile()
    _NC_CACHE = nc
    return nc


def kernel(input_ri, cof, Ns):
    global LAST_RESULT
    assert int(Ns) == S, f"kernel hardcodes S={S}, got Ns={Ns}"
    input_ri = np.ascontiguousarray(np.asarray(input_ri, dtype=np.float32))
    cof = np.ascontiguousarray(np.asarray(cof, dtype=np.float32))
    assert input_ri.shape == (N, P, S * M, 2) and cof.shape == (N, P, L)

    cc_base, wc_base = _host_consts()
    nc = _build()

    in_maps = []
    for c in range(NCORES):
        n0 = c * NLOC
        x_sh = np.ascontiguousarray(
            input_ri[n0:n0 + NLOC].reshape(NP, S, M, 2))
        cof_sh = np.ascontiguousarray(cof[n0:n0 + NLOC].reshape(NP, L))
        cc = np.concatenate([cc_base, cof_sh.reshape(NP * L, 1)], axis=1)
        wc = np.concatenate([wc_base, cof_sh.T], axis=1)
        in_maps.append({
            "x": x_sh,
            "cc": np.ascontiguousarray(cc),
            "wc": np.ascontiguousarray(wc),
        })

    res = bass_utils.run_bass_kernel_spmd(
        nc, in_maps, core_ids=list(range(NCORES)), trace=TRACE)
    LAST_RESULT = res

    out_ri = np.concatenate(
        [res.results[c]["out"].reshape(NLOC, P, S * M, 2) for c in range(NCORES)],
        axis=0)
    h_ri = np.concatenate(
        [res.results[c]["hout"].reshape(NLOC, P, M, 2) for c in range(NCORES)],
        axis=0)
    return out_ri, h_ri
